# revision 31
# baseline (speedup 1.0000x reference)
"""AU-guided attention fusion kernel for 8 Trainium2 NeuronCores.

Strategy (pure data parallel, batch sharded 8 ways, weights replicated):

The reference module is algebraically restructured on the host so the device
does only dense work in a [features-on-partitions, batch-on-free] layout:

  - au_tokens = LN(au*w_tok + b_tok + pos) is affine in the scalar t=au[b,a],
    so k/v rows collapse to (t*u + V_a)/s(t,a) + c with tiny per-(a) constant
    vectors (kills the B*A*H*H in_proj matmuls entirely).
  - The 1-query attention (A=17 keys) reduces to exp/softmax on a [68, Nb]
    tile (4 heads x 17 slots) plus small stationary matmuls over 68 rows.
  - The softmax and LayerNorm denominators ride PE matmuls (ones-reduce and
    K=1 broadcast); 1/sqrt goes through exp(-0.5*ln(x)) so the whole kernel
    uses one ACT table set (natural_log_exp_and_others, pinned at compile
    time); sigmoid is 1/(1+exp(-y)) with the fast DVE reciprocal.
  - q/out/gate projections are folded host-side (wq@w_q, Wg2@w_out etc.) so
    gate needs one [B,768]x[768,768] matmul plus cheap [B,128] tails.

Matmuls run in bf16 (1 PE cycle/row, fast weight loads); the residual
combine out = x + g*(af - x) uses the fp32 copy of x so input rounding never
touches the skip path. Inputs are fed transposed ([D, Bc] per core); the
host transposes back. Independent consumers of one producer are split
across engines (Es on GPSIMD, Et on DVE) so the in-order engine queues run
them in parallel; measured ~206 us on 8 cores at 3.4e-3 relative error.
"""

import numpy as np

import concourse.bass as bass
import concourse.tile as tile
from concourse import bacc, mybir
from concourse.bass import ts
from concourse.bass_utils import run_bass_kernel_spmd

B, D, A, H, NH = 32768, 768, 17, 128, 4
DH = H // NH
NCORES = 8
BC = B // NCORES          # 4096 rows per core
NB = 512                  # batch columns per tile (matmul free dim)
NT = BC // NB             # 8 tiles per core
KD = D // H               # 6 feature blocks of 128
A4 = NH * A               # 68
EPS = 1e-5

F32 = mybir.dt.float32
BF16 = mybir.dt.bfloat16
AF = mybir.ActivationFunctionType
OP = mybir.AluOpType

ACT_SET = "natural_log_exp_and_others"


def _pin_act_tables():
    """Force every activation into the one table set that has exp AND ln,
    so the kernel pays a single ACT_TABLE_LOAD instead of thrashing between
    exp_and_others / natural_log every tile. Returns the previous function
    so the caller can restore it."""
    used = {AF.Exp, AF.Ln, AF.Square, AF.Identity, AF.Copy}
    prev = bacc.get_activation_tables

    def patched(arch):
        tabs = dict(prev(arch))
        return {
            name: (set(fns) if name == ACT_SET else set(fns) - used)
            for name, fns in tabs.items()
        }

    bacc.get_activation_tables = patched
    return prev


def build_bass():
    nc = bacc.Bacc("TRN2", target_bir_lowering=False, debug=False,
                   num_devices=NCORES)

    xT = nc.dram_tensor("xT", [D, BC], F32, kind="ExternalInput")
    xTb = nc.dram_tensor("xTb", [D, BC], BF16, kind="ExternalInput")
    auR = nc.dram_tensor("auR", [A4, BC], F32, kind="ExternalInput")
    wqeT = nc.dram_tensor("wqeT", [D, H], BF16, kind="ExternalInput")
    mamb = nc.dram_tensor("mamb", [H, 2 * A4], BF16, kind="ExternalInput")
    g4 = nc.dram_tensor("g4", [A4, 4 * H], BF16, kind="ExternalInput")
    waoT = nc.dram_tensor("waoT", [H, H], BF16, kind="ExternalInput")
    woutT = nc.dram_tensor("woutT", [H, D], BF16, kind="ExternalInput")
    w2eT = nc.dram_tensor("w2eT", [H, D], BF16, kind="ExternalInput")
    wg1T = nc.dram_tensor("wg1T", [D, D], BF16, kind="ExternalInput")
    negI = nc.dram_tensor("negI", [H, H], BF16, kind="ExternalInput")
    # cvec columns: 0 bq_eff | 1 g_aln | 2 bhat*g_aln | 3:9 bfin_j | 9:15 -bg_j
    cvec = nc.dram_tensor("cvec", [H, 16], F32, kind="ExternalInput")
    # c68 columns: 0 sqrt(alpha) | 1 k_a rep | 2 sqrt(alpha)*h_a rep | 3 pad
    c68 = nc.dram_tensor("c68", [A4, 4], F32, kind="ExternalInput")
    wvard = nc.dram_tensor("wvard", [H, 1], BF16, kind="ExternalInput")
    onesd = nc.dram_tensor("onesd", [1, H], BF16, kind="ExternalInput")

    outT = nc.dram_tensor("outT", [D, BC], F32, kind="ExternalOutput")

    xT_r = xT[:, :].rearrange("(i p) n -> p i n", p=H)
    xTb_r = xTb[:, :].rearrange("(i p) n -> p i n", p=H)
    wg1_r = wg1T[:, :].rearrange("(i p) m -> p i m", p=H)
    wqe_r = wqeT[:, :].rearrange("(i p) m -> p i m", p=H)
    outT_r = outT[:, :].rearrange("(j p) n -> p j n", p=H)

    with tile.TileContext(nc) as tc:
        with (
            tc.tile_pool(name="consts", bufs=1) as cons,
            tc.tile_pool(name="vis", bufs=3) as pvis,
            tc.tile_pool(name="p68", bufs=3) as p68,
            tc.tile_pool(name="pq", bufs=3) as pq,
            tc.tile_pool(name="pblk", bufs=3) as pblk,
            tc.tile_pool(name="pout", bufs=2) as pout,
            tc.tile_pool(name="psum", bufs=1, space="PSUM") as ps,
        ):
            # ---- constants into SBUF (once) ----
            wq_sb = cons.tile([H, KD, H], BF16)
            nc.sync.dma_start(out=wq_sb, in_=wqe_r)
            mamb_sb = cons.tile([H, 2 * A4], BF16)
            nc.sync.dma_start(out=mamb_sb, in_=mamb[:, :])
            g4_sb = cons.tile([A4, 4 * H], BF16)
            nc.sync.dma_start(out=g4_sb, in_=g4[:, :])
            wao_sb = cons.tile([H, H], BF16)
            nc.sync.dma_start(out=wao_sb, in_=waoT[:, :])
            wout_sb = cons.tile([H, D], BF16)
            nc.sync.dma_start(out=wout_sb, in_=woutT[:, :])
            w2e_sb = cons.tile([H, D], BF16)
            nc.sync.dma_start(out=w2e_sb, in_=w2eT[:, :])
            negI_sb = cons.tile([H, H], BF16)
            nc.sync.dma_start(out=negI_sb, in_=negI[:, :])
            cvec_sb = cons.tile([H, 16], F32)
            nc.sync.dma_start(out=cvec_sb, in_=cvec[:, :])
            c68_sb = cons.tile([A4, 4], F32)
            nc.sync.dma_start(out=c68_sb, in_=c68[:, :])
            wvar_sb = cons.tile([H, 1], BF16)
            nc.sync.dma_start(out=wvar_sb, in_=wvard[:, :])
            ones_sb = cons.tile([1, H], BF16)
            nc.sync.dma_start(out=ones_sb, in_=onesd[:, :])
            eps_sb = cons.tile([1, 1], F32)
            nc.vector.memset(eps_sb, EPS)
            wg1_sb = cons.tile([H, KD, D], BF16)
            nc.sync.dma_start(out=wg1_sb, in_=wg1_r)

            for t in range(NT):
                cols = ts(t, NB)

                vis = pvis.tile([H, KD, NB], F32, tag="vis")
                nc.sync.dma_start(out=vis, in_=xT_r[:, :, cols])
                visb = pvis.tile([H, KD, NB], BF16, tag="visb")
                nc.sync.dma_start(out=visb, in_=xTb_r[:, :, cols])
                au = p68.tile([A4, NB], F32, tag="au")
                nc.sync.dma_start(out=au, in_=auR[:, cols])

                # ---- r = 1/sqrt(alpha*(t+h)^2 + k) ----
                sq = p68.tile([A4, NB], F32, tag="sq")
                nc.scalar.activation(sq, au, AF.Square,
                                     scale=c68_sb[:, 0:1],
                                     bias=c68_sb[:, 2:3])
                lq = p68.tile([A4, NB], F32, tag="lq")
                nc.scalar.activation(lq, sq, AF.Ln, bias=c68_sb[:, 1:2])
                r = p68.tile([A4, NB], F32, tag="r")
                nc.scalar.activation(r, lq, AF.Exp, scale=-0.5)
                tr = p68.tile([A4, NB], F32, tag="tr")
                nc.gpsimd.tensor_mul(tr, r, au)

                # ---- q = Wq_eff @ x + bq ----
                psq = ps.tile([H, NB], F32, tag="pA", bufs=2)
                for i in range(KD):
                    nc.tensor.matmul(psq, wq_sb[:, i, :], visb[:, i, :],
                                     start=(i == 0), stop=(i == KD - 1))
                qs = pq.tile([H, NB], BF16, tag="qs")
                nc.scalar.activation(qs, psq, AF.Identity,
                                     bias=cvec_sb[:, 0:1])

                # ---- scores -> E, Es, Et ----
                psa = ps.tile([A4, NB], F32, tag="ps68", bufs=2)
                nc.tensor.matmul(psa, mamb_sb[:, 0:A4], qs)
                psb = ps.tile([A4, NB], F32, tag="ps68", bufs=2)
                nc.tensor.matmul(psb, mamb_sb[:, A4:2 * A4], qs)
                x1 = p68.tile([A4, NB], F32, tag="x1")
                nc.vector.tensor_mul(x1, au, psa)
                x2 = p68.tile([A4, NB], F32, tag="x2")
                nc.vector.tensor_add(x2, x1, psb)
                sc = p68.tile([A4, NB], F32, tag="sc")
                nc.vector.tensor_mul(sc, x2, r)
                Ee = p68.tile([A4, NB], BF16, tag="Ee")
                nc.scalar.activation(Ee, sc, AF.Exp)
                Es = p68.tile([A4, NB], BF16, tag="Es")
                nc.gpsimd.tensor_mul(Es, Ee, r)
                Et = p68.tile([A4, NB], BF16, tag="Et")
                nc.vector.tensor_mul(Et, Ee, tr)

                # ---- combine: numer / denom ----
                pdn = ps.tile([H, NB], F32, tag="pdg", bufs=4)
                nc.tensor.matmul(pdn, g4_sb[:, 3 * H:4 * H], Ee)
                pnm = ps.tile([H, NB], F32, tag="pA", bufs=2)
                nc.tensor.matmul(pnm, g4_sb[:, 2 * H:3 * H], Ee,
                                 start=True, stop=False)
                nc.tensor.matmul(pnm, g4_sb[:, H:2 * H], Et,
                                 start=False, stop=False)
                nc.tensor.matmul(pnm, g4_sb[:, 0:H], Es,
                                 start=False, stop=True)
                rd = pq.tile([H, NB], F32, tag="rd")
                nc.vector.reciprocal_approx_fast(out=rd, in_=pdn)
                ctx = pq.tile([H, NB], BF16, tag="ctx")
                nc.vector.tensor_mul(ctx, pnm, rd)

                # ---- attn out proj (centered) + LN ----
                pao = ps.tile([H, NB], F32, tag="pA", bufs=2)
                nc.tensor.matmul(pao, wao_sb, ctx)
                chg = pq.tile([H, NB], F32, tag="chg")
                nc.scalar.activation(chg, pao, AF.Identity,
                                     scale=cvec_sb[:, 1:2],
                                     bias=cvec_sb[:, 2:3])
                c2 = pq.tile([H, NB], BF16, tag="c2")
                nc.scalar.activation(c2, chg, AF.Square)
                pvar = ps.tile([1, NB], F32, tag="ps68", bufs=2)
                nc.tensor.matmul(pvar, wvar_sb[:, 0:1], c2)
                lv = pq.tile([1, NB], F32, tag="lv")
                nc.scalar.activation(lv, pvar, AF.Ln, bias=eps_sb[:, 0:1])
                rv = pq.tile([1, NB], BF16, tag="rv")
                nc.scalar.activation(rv, lv, AF.Exp, scale=-0.5)
                # block 0's visual-only matmuls are ready now; emit them
                # before prs so the PE isn't stalled behind the rv wait
                pg0 = ps.tile([H, NB], F32, tag="pdg", bufs=4)
                for i in range(KD):
                    nc.tensor.matmul(pg0, wg1_sb[:, i, 0:H], visb[:, i, :],
                                     start=(i == 0), stop=False)
                prs = ps.tile([H, NB], F32, tag="pdg", bufs=4)
                nc.tensor.matmul(prs, ones_sb[0:1, :], rv)
                lng = pq.tile([H, NB], BF16, tag="lng")
                nc.vector.tensor_mul(lng, chg, prs)

                # ---- output feature blocks ----
                ot = pout.tile([H, KD, NB], F32, tag="ot")
                for j in range(KD):
                    jb = ts(j, H)
                    if j == 0:
                        pg = pg0
                    else:
                        pg = ps.tile([H, NB], F32, tag="pdg", bufs=4)
                        for i in range(KD):
                            nc.tensor.matmul(pg, wg1_sb[:, i, jb],
                                             visb[:, i, :],
                                             start=(i == 0), stop=False)
                    nc.tensor.matmul(pg, w2e_sb[:, jb], lng,
                                     start=False, stop=True)
                    ej = pblk.tile([H, NB], F32, tag="ej")
                    nc.scalar.activation(ej, pg, AF.Exp, scale=-1.0,
                                         bias=cvec_sb[:, 9 + j:10 + j])
                    den = pblk.tile([H, NB], F32, tag="den")
                    nc.scalar.activation(den, ej, AF.Identity, bias=1.0)
                    gj = pblk.tile([H, NB], F32, tag="gj")
                    nc.vector.reciprocal_approx_fast(out=gj, in_=den)

                    pd = ps.tile([H, NB], F32, tag="pdg", bufs=4)
                    nc.tensor.matmul(pd, wout_sb[:, jb], lng,
                                     start=True, stop=False)
                    nc.tensor.matmul(pd, negI_sb, visb[:, j, :],
                                     start=False, stop=True)
                    mj = pblk.tile([H, NB], F32, tag="mj")
                    nc.vector.scalar_tensor_tensor(
                        mj, pd, cvec_sb[:, 3 + j:4 + j], gj,
                        op0=OP.add, op1=OP.mult)
                    nc.gpsimd.tensor_add(ot[:, j, :], vis[:, j, :], mj)

                nc.sync.dma_start(out=outT_r[:, :, cols], in_=ot)

    prev = _pin_act_tables()
    try:
        nc.finalize()
    finally:
        bacc.get_activation_tables = prev
    return nc


def host_constants(inputs):
    f = lambda k: np.asarray(inputs[k], np.float64)
    w = f("w_tok")[:, 0]
    pos0 = f("pos")[0]
    cA = f("b_tok")[None, :] + pos0
    mw = w.mean()
    wp = w - mw
    mc = cA.mean(axis=1, keepdims=True)
    cp = cA - mc
    alpha = (wp ** 2).mean()
    beta = 2.0 * (wp[None, :] * cp).mean(axis=1)
    gamma = (cp ** 2).mean(axis=1)
    h_a = beta / (2 * alpha)
    k_a = gamma + EPS - beta ** 2 / (4 * alpha)

    g_au = f("g_auln")
    b_au = f("b_auln")
    wg = wp * g_au
    cg = cp * g_au[None, :]
    w_in = f("w_in")
    b_in = f("b_in")
    wq_, wk_, wv_ = w_in[:H], w_in[H:2 * H], w_in[2 * H:]
    bq_, bk_, bv_ = b_in[:H], b_in[H:2 * H], b_in[2 * H:]
    u_k = wk_ @ wg
    Vk = cg @ wk_.T
    u_v = wv_ @ wg
    Vv = cg @ wv_.T
    cv = wv_ @ b_au + bv_
    scale = 1.0 / np.sqrt(DH)
    Wq_eff = (wq_ @ f("w_q")) * scale
    bq_eff = (wq_ @ f("b_q") + bq_) * scale

    head = np.arange(H) // DH
    colhead = np.repeat(np.arange(NH), A)
    cola = np.tile(np.arange(A), NH)
    mask = (head[:, None] == colhead[None, :]).astype(np.float64)
    MA = u_k[:, None] * mask
    MB = Vk[cola, :].T * mask
    maskT = mask.T
    Gv = Vv[cola, :] * maskT
    Gu = u_v[None, :] * maskT
    Gc = cv[None, :] * maskT
    Gd = maskT

    w_ao = f("w_ao")
    b_ao = f("b_ao")
    m_ao = w_ao.mean(axis=0)
    mb_ao = b_ao.mean()
    What = w_ao - m_ao[None, :]
    bhat = b_ao - mb_ao
    g_aln = f("g_aln")
    b_aln = f("b_aln")
    assert (np.abs(g_aln) > 1e-6).all(), "zero LN gain not supported"
    wvar = 1.0 / (H * g_aln ** 2)
    w_out = f("w_out")
    b_out = f("b_out")
    bfin = w_out @ b_aln + b_out
    w_gate = f("w_gate")
    b_gate = f("b_gate")
    Wg1 = w_gate[:, :D]
    Wg2 = w_gate[:, D:]
    W2eff = Wg2 @ w_out
    bg_eff = b_gate + Wg2 @ b_out + W2eff @ b_aln

    bf = mybir.dt.np(BF16)
    c = lambda x: np.ascontiguousarray(np.asarray(x, np.float32))
    cb = lambda x: np.ascontiguousarray(np.asarray(x, np.float32).astype(bf))
    cvec = np.zeros((H, 16), np.float64)
    cvec[:, 0] = bq_eff
    cvec[:, 1] = g_aln
    cvec[:, 2] = bhat * g_aln
    for j in range(KD):
        cvec[:, 3 + j] = bfin[j * H:(j + 1) * H]
        cvec[:, 9 + j] = -bg_eff[j * H:(j + 1) * H]
    c68 = np.zeros((A4, 4), np.float64)
    c68[:, 0] = np.sqrt(alpha)
    c68[:, 1] = k_a[cola]
    c68[:, 2] = np.sqrt(alpha) * np.tile(h_a, NH)

    return {
        "wqeT": cb(Wq_eff.T),
        "mamb": cb(np.concatenate([MA, MB], axis=1)),
        "g4": cb(np.concatenate([Gv, Gu, Gc, Gd], axis=1)),
        "waoT": cb(What.T),
        "woutT": cb(w_out.T),
        "w2eT": cb(W2eff.T),
        "wg1T": cb(Wg1.T),
        "negI": cb(-np.eye(H)),
        "cvec": c(cvec),
        "c68": c(c68),
        "wvard": cb(wvar[:, None]),
        "onesd": cb(np.ones((1, H))),
    }


_BUILT = {}


def _get_nc():
    if "nc" not in _BUILT:
        _BUILT["nc"] = build_bass()
    return _BUILT["nc"]


def _run(inputs, trace=False):
    vf = np.ascontiguousarray(np.asarray(inputs["visual_feat"], np.float32))
    af = np.ascontiguousarray(np.asarray(inputs["au_feat"], np.float32))
    consts = host_constants(inputs)
    bf = mybir.dt.np(BF16)

    in_maps = []
    for ci in range(NCORES):
        sl = slice(ci * BC, (ci + 1) * BC)
        m = dict(consts)
        xTc = np.ascontiguousarray(vf[sl].T)
        m["xT"] = xTc
        m["xTb"] = np.ascontiguousarray(xTc.astype(bf))
        auT = np.ascontiguousarray(af[sl].T)         # [A, BC]
        m["auR"] = np.ascontiguousarray(np.tile(auT, (NH, 1)))
        in_maps.append(m)

    nc = _get_nc()
    res = run_bass_kernel_spmd(nc, in_maps, list(range(NCORES)), trace=trace)
    out = np.empty((B, D), np.float32)
    for ci in range(NCORES):
        out[ci * BC:(ci + 1) * BC] = res.results[ci]["outT"].T
    return out, res


def kernel(**inputs):
    out, _ = _run(inputs, trace=False)
    return out
